# revision 18
# baseline (speedup 1.0000x reference)
"""Trainium2 Bass kernel for nn_AttentionBlock (B=2, N=2048, dim=1024, 16 heads x 64).

Sharding: 8 cores = 2 batches x 4 head-groups (4 heads per core, tensor-parallel
over heads for qkv/attention; the to_out projection is computed as per-core,
per-i-tile partial sums gathered and added on host).

Per-core device program (SPMD, identical shapes on every core):
  inputs (bf16, pre-transposed on host):
    xT [1024, 2048], wqT/wkT/wvT [1024, 256], woT [256, 1024]
  outputs (f32): y0, y1 [2048, 1024] — partial projections for i-tile 0
    (heads 0,1) and i-tile 1 (heads 2,3); host adds them.

Structure: per head-pair (= i-tile) and 512-wide q-window, a 16-step loop over
k-tiles computes S^T for both heads concurrently (row-groups 0-63 / 64-127 of
the PE array, one [128, 2, 512] PSUM tile), one exp ACTIVATE (FD=1024, fused
1/8 scale, PSUM->SBUF bf16), then P^T @ [V|1] accumulates O^T plus softmax row
sums in PSUM. Normalize+project for each step is emitted one step later so its
reciprocal DMA chain never stalls the PE queue. Matmuls are bf16 with fp32
accumulation; softmax skips max-subtraction (logits ~N(0,1), exp safe in fp32).
"""

import ml_dtypes
import numpy as np

import concourse.bass as bass
import concourse.mybir as mybir
import concourse.tile as tile
from concourse.bass_utils import run_bass_kernel_spmd

B = 2
N = 2048
D = 1024
H = 16
DH = 64
HPC = 4  # heads per core
NCORES = 8
HB = HPC * DH  # 256: head-block width per core
NKT = N // 128  # 16 k-tiles
NW = 4  # 512-wide q-windows

f32 = mybir.dt.float32
f32r = mybir.dt.float32r
bf16 = mybir.dt.bfloat16
EXP = mybir.ActivationFunctionType.Exp

_WAIT_CAP = 1


def _split_excess_waits(nc):
    """The walrus build in this container rejects instructions carrying more
    than a couple of sync-wait commands ("Too many sync wait commands" in
    CoreV3GenImpl setupSyncWait). Tile's semaphore assignment freely attaches
    several waits to one instruction. Hoist the excess onto dedicated
    single-wait NOPs inserted just before the instruction on the same engine
    (program order on that engine preserves the wait-before-execute
    semantics)."""
    f = nc.m.functions[0]
    for blk in f.blocks:
        out = []
        changed = False
        for inst in blk.instructions:
            si = inst.sync_info
            waits = list(si.on_wait) if si is not None and si.on_wait else []
            if len(waits) > _WAIT_CAP:
                changed = True
                for j, w in enumerate(waits[: -_WAIT_CAP]):
                    nop = mybir.InstNoOp(
                        name=f"{inst.name}-ws{j}",
                        engine=inst.engine,
                        sync_info=mybir.SyncInfo(on_wait=[w], on_update=[]),
                        bass_nofuse=True,
                    )
                    nc.register_instruction(nop)
                    out.append(nop)
                si.on_wait = waits[-_WAIT_CAP:]
            out.append(inst)
        if changed:
            blk.instructions = out


def _r(ap):
    return ap.bitcast(f32r)


def _build_nc():
    nc = bass.Bass()
    xT_d = nc.dram_tensor("xT", [D, N], bf16, kind="ExternalInput")
    wqT_d = nc.dram_tensor("wqT", [D, HB], bf16, kind="ExternalInput")
    wkT_d = nc.dram_tensor("wkT", [D, HB], bf16, kind="ExternalInput")
    wvT_d = nc.dram_tensor("wvT", [D, HB], bf16, kind="ExternalInput")
    woT_d = nc.dram_tensor("woT", [HB, D], bf16, kind="ExternalInput")
    y_ds = [
        nc.dram_tensor(f"y{it}", [N, D], f32, kind="ExternalOutput") for it in range(2)
    ]

    with tile.TileContext(nc) as tc:
        with (
            tc.tile_pool(name="main", bufs=1) as main,
            tc.tile_pool(name="ptp", bufs=3) as ptp,
            tc.tile_pool(name="ysp", bufs=3) as ysp,
            tc.tile_pool(name="spp", bufs=2) as spp,
            tc.tile_pool(name="drm", bufs=2, space="DRAM") as drm,
            tc.tile_pool(name="aux", bufs=1, space="PSUM") as aux,
            tc.tile_pool(name="stp", bufs=2, space="PSUM") as stp,
            tc.tile_pool(name="otp", bufs=1, space="PSUM") as otp,
        ):
            # persistent tensors
            qT = main.tile([128, 2, N], bf16)  # row d = it*128+p
            kT = main.tile([128, 2, N], bf16)
            vaug = main.tile([128, NKT, HPC, DH + 1], bf16)  # [k%128, k//128, h, d|1]
            ocat = main.tile([128, 2, N], bf16)  # row i = it*128+p
            wo = main.tile([128, 2, D], bf16)
            xt = main.tile([128, 8, N], bf16)
            wq = main.tile([128, 8, HB], bf16)
            wk = main.tile([128, 8, HB], bf16)
            wv = main.tile([128, 8, HB], bf16)

            ones_t = main.tile([128, 1], bf16)
            nc.vector.memset(ones_t[:], 1.0)
            nc.vector.tensor_copy(
                vaug[:, :, :, DH : DH + 1],
                ones_t[:, :, None, None].to_broadcast([128, NKT, HPC, 1]),
            )
            nc.sync.dma_start(wk[:], wkT_d.rearrange("(e p) c -> p e c", p=128))
            nc.sync.dma_start(wq[:], wqT_d.rearrange("(e p) c -> p e c", p=128))
            for c in range(4):
                nc.sync.dma_start(
                    xt[:, 2 * c : 2 * c + 2],
                    xT_d[c * 256 : (c + 1) * 256].rearrange(
                        "(e p) n -> p e n", p=128
                    ),
                )
            nc.gpsimd.dma_start(wv[:], wvT_d.rearrange("(e p) c -> p e c", p=128))
            nc.gpsimd.dma_start(wo[:], woT_d.rearrange("(e p) o -> p e o", p=128))

            # ---- projection-group emitters (each: 8 accumulating matmuls) ----
            def emit_qk_group(dst, w, it, q4):
                ps = aux.tile([128, 512], f32, tag="qkv")
                for eo in range(8):
                    nc.tensor.matmul(
                        ps,
                        lhsT=w[:, eo, it * 128 : (it + 1) * 128],
                        rhs=xt[:, eo, q4 * 512 : (q4 + 1) * 512],
                        start=(eo == 0),
                        stop=(eo == 7),
                    )
                nc.vector.tensor_copy(dst[:, it, q4 * 512 : (q4 + 1) * 512], ps)

            def emit_v_group(nt):
                ps_full = aux.tile([128, 512], f32, tag="qkv", name=f"vps{nt}")
                ps = ps_full[:, 0:HB]
                for eo in range(8):
                    nc.tensor.matmul(
                        ps,
                        lhsT=xt[:, eo, nt * 128 : (nt + 1) * 128],
                        rhs=wv[:, eo, :],
                        start=(eo == 0),
                        stop=(eo == 7),
                    )
                nc.vector.tensor_copy(
                    vaug[:, nt, :, 0:DH], ps.rearrange("p (h d) -> p h d", h=HPC)
                )

            # upfront groups, ordered by when step 0 needs them; they run on
            # the PE while the input DMAs are still streaming in.
            emit_qk_group(kT, wk, 0, 0)
            emit_qk_group(qT, wq, 0, 0)
            for q4 in range(1, 4):
                emit_qk_group(kT, wk, 0, q4)
            for nt in range(6):
                emit_v_group(nt)

            # remaining projection groups, drip-fed into attention steps at
            # ~1 group per 5 k-tiles so the exp stream never starves and the
            # single qkv PSUM slot never backs up. Tile tracks dependencies in
            # emission order, so every group is EMITTED strictly before its
            # consumer (step s consumes qT[it=s//4] window q4=s%4 and, within
            # its own kt loop, kT[it] window q4=kt//4; kT it1 from step 4 on).
            def qk(dst, w, it, q4):
                return lambda: emit_qk_group(dst, w, it, q4)

            drip = {
                0: {nt - 2: (lambda nt=nt: emit_v_group(nt)) for nt in range(6, 16)},
                1: {2: qk(qT, wq, 0, 2), 7: qk(kT, wk, 1, 0), 12: qk(kT, wk, 1, 1)},
                2: {2: qk(qT, wq, 0, 3), 7: qk(kT, wk, 1, 2), 12: qk(kT, wk, 1, 3)},
                3: {2: qk(qT, wq, 1, 0), 9: qk(qT, wq, 1, 1)},
                4: {5: qk(qT, wq, 1, 2)},
                5: {5: qk(qT, wq, 1, 3)},
            }
            drip[0][14] = qk(qT, wq, 0, 1)

            # ---- attention steps ----
            steps = [(it, w) for it in range(2) for w in range(NW)]

            def att_step(step_idx, it, w, mid_a=None, mid_b=None):
                q0 = w * 512
                h_lo, h_hi = 2 * it, 2 * it + 1
                ot_lo = otp.tile([128, 512], f32, tag="otlo")
                ot_hi = otp.tile([128, 512], f32, tag="othi")
                fillers = dict(drip.get(step_idx, {}))
                prev_pv = None
                for kt in range(NKT):
                    if mid_a is not None and kt == 1:
                        mid_a()
                        mid_a = None
                    if mid_b is not None and kt == 8:
                        mid_b()
                        mid_b = None
                    f = fillers.pop(kt, None)
                    if f is not None:
                        f()
                    st2 = stp.tile([128, 2, 512], f32, tag="st")
                    for s in range(2):
                        nc.tensor.matmul(
                            st2[:, s, :],
                            lhsT=kT[
                                s * 64 : s * 64 + 64, it, kt * 128 : (kt + 1) * 128
                            ],
                            rhs=qT[s * 64 : s * 64 + 64, it, q0 : q0 + 512],
                            start=True,
                            stop=True,
                        )
                    pt2 = ptp.tile([128, 2, 512], bf16, tag="pt")
                    nc.scalar.activation(
                        pt2.rearrange("p s q -> p (s q)"),
                        st2.rearrange("p s q -> p (s q)"),
                        EXP,
                        scale=0.125,
                    )
                    for s, ot in ((0, ot_lo), (1, ot_hi)):
                        nc.tensor.matmul(
                            ot[0:65, :],
                            lhsT=vaug[:, kt, 2 * it + s, :],
                            rhs=pt2[:, s, :],
                            start=(kt == 0),
                            stop=(kt == NKT - 1),
                        )
                if mid_a is not None:
                    mid_a()
                if mid_b is not None:
                    mid_b()
                for kt in sorted(fillers):
                    fillers[kt]()
                # evacuate: unnormalized O^T (cast to bf16) + row sums
                sst = spp.tile([1, 1024], f32, tag="sst")
                for s, ot in ((0, ot_lo), (1, ot_hi)):
                    nc.vector.tensor_copy(
                        ocat[s * 64 : s * 64 + 64, it, q0 : q0 + 512], ot[0:64, :]
                    )
                    nc.vector.tensor_copy(
                        sst[0:1, s * 512 : (s + 1) * 512], ot[64:65, :]
                    )
                return sst

            def np_chain(sst):
                # reciprocal of both heads' row sums, then a DRAM bounce so a
                # broadcast DMA can replicate each head's 512 values across
                # its 64 ocat rows. Pure DMA/DVE: emitted early so the
                # latency hides under the next attention step.
                rr = spp.tile([1, 1024], f32, tag="rr")
                nc.vector.reciprocal(rr, sst)
                rtmp = drm.tile([1, 1024], f32, tag="rtmp")
                nc.sync.dma_start(rtmp, rr)
                bc32 = spp.tile([128, 512], f32, tag="bc32")
                rv = rtmp.rearrange("a (s q) -> (a s) q", s=2)
                for s in range(2):
                    nc.sync.dma_start(
                        bc32[s * 64 : (s + 1) * 64, :],
                        rv[s : s + 1, :].to_broadcast([64, 512]),
                    )
                return bc32

            def np_project(it, w, bc32):
                q0 = w * 512
                osl = ocat[:, it, q0 : q0 + 512]
                nc.vector.tensor_mul(osl, osl, bc32)
                # output projection for this (i-tile, window): partial sums
                for qt in range(4):
                    r0 = q0 + qt * 128
                    for oc in range(2):
                        yp = aux.tile(
                            [128, 512], f32, tag=("np" if (qt + oc) % 2 else "qkv"),
                            name=f"yp{it}_{w}_{qt}_{oc}",
                        )
                        nc.tensor.matmul(
                            yp,
                            lhsT=ocat[:, it, r0 : r0 + 128],
                            rhs=wo[:, it, oc * 512 : (oc + 1) * 512],
                            start=True,
                            stop=True,
                        )
                        ys = ysp.tile([128, 512], f32, tag="ys")
                        nc.vector.tensor_copy(ys, yp)
                        nc.gpsimd.dma_start(
                            y_ds[it][r0 : r0 + 128, oc * 512 : (oc + 1) * 512], ys
                        )

            # each step's normalize+project runs split across the following
            # step: the reciprocal DMA chain right after the sums land (kt 1),
            # the PE part once the chain has resolved (kt 8)
            prev_a = prev_b = None
            for idx, (it, w) in enumerate(steps):
                sst = att_step(idx, it, w, mid_a=prev_a, mid_b=prev_b)
                prev_a = lambda it=it, w=w, sst=sst: setattr(
                    np_chain, "_bc", np_chain(sst)
                )
                prev_b = lambda it=it, w=w: np_project(it, w, np_chain._bc)
            prev_a()
            prev_b()  # the last step's norm+project is the kernel tail

    _split_excess_waits(nc)
    return nc


_CACHED_NC = None


def _get_nc():
    global _CACHED_NC
    if _CACHED_NC is None:
        _CACHED_NC = _build_nc()
    return _CACHED_NC


def _make_in_maps(x, w_qkv):
    b16 = ml_dtypes.bfloat16

    def c(a):
        return np.ascontiguousarray(a.astype(b16))

    in_maps = []
    xT = [c(x[b].T) for b in range(B)]
    for core in range(NCORES):
        b = core // (NCORES // B)
        hb = core % (NCORES // B)
        rows = slice(hb * HB, (hb + 1) * HB)
        wq = c(w_qkv[0 * D : 1 * D][rows].T)
        wk = c(w_qkv[1 * D : 2 * D][rows].T)
        wv = c(w_qkv[2 * D : 3 * D][rows].T)
        in_maps.append({"xT": xT[b], "wqT": wq, "wkT": wk, "wvT": wv})
    return in_maps


def kernel(x, w_qkv, w_out, b_out, _trace=False, _trace_kwargs=None):
    x = np.asarray(x, dtype=np.float32)
    w_qkv = np.asarray(w_qkv, dtype=np.float32)
    w_out = np.asarray(w_out, dtype=np.float32)
    b_out = np.asarray(b_out, dtype=np.float32)

    in_maps = _make_in_maps(x, w_qkv)
    for core in range(NCORES):
        hb = core % (NCORES // B)
        woT = np.ascontiguousarray(
            w_out[:, hb * HB : (hb + 1) * HB].T.astype(ml_dtypes.bfloat16)
        )
        in_maps[core]["woT"] = woT

    nc = _get_nc()
    kwargs = {}
    if _trace:
        kwargs["trace"] = True
        if _trace_kwargs:
            kwargs.update(_trace_kwargs)
    res = run_bass_kernel_spmd(nc, in_maps, core_ids=list(range(NCORES)), **kwargs)

    out = np.zeros((B, N, D), dtype=np.float32)
    for core in range(NCORES):
        b = core // (NCORES // B)
        out[b] += res.results[core]["y0"]
        out[b] += res.results[core]["y1"]
    out += b_out[None, None, :]
    kernel._last_result = res
    return out
